# revision 1
# baseline (speedup 1.0000x reference)
# GATv2 two-layer GNN on 8 TRN2 cores.
# Design: node-major tables in 4 int16-addressable windows; per-window ELL slot
# groups gathered with the custom dma_gather; softmax without max-subtraction
# (cancels exactly); per-region partial sums combined via a second gather.
import numpy as np
import concourse.bass as bass
import concourse.bacc as bacc
import concourse.mybir as mybir
from concourse.tile import TileContext

F32 = mybir.dt.float32
I16 = mybir.dt.int16
AL = mybir.AluOpType
AF = mybir.ActivationFunctionType
AX = mybir.AxisListType

PADC = 1.0e4  # magnitude of the pad-row sentinel (-PADC * sign(att))


def _wrap16(arr):
    # flat int array (len % 16 == 0) -> [128, len/16] wrapped+replicated int16
    a = np.asarray(arr, np.int16).reshape(-1, 16).T  # [16, L/16]
    return np.tile(a, (8, 1))


class Plan:
    pass


def build_plan(edge_index, n_nodes, n_cores=8, GB=8, CB=7):
    """Host-side graph preprocessing. edge_index [2, E] (no self loops yet)."""
    p = Plan()
    P = 128
    NWIN = n_cores // 2
    src = np.asarray(edge_index[0], np.int64)
    dst = np.asarray(edge_index[1], np.int64)
    loops = np.arange(n_nodes, dtype=np.int64)
    src = np.concatenate([src, loops])
    dst = np.concatenate([dst, loops])

    deg = np.bincount(dst, minlength=n_nodes)  # in-degree incl self loop
    order = np.argsort(-deg, kind="stable")    # degree-sorted deal
    rank = np.empty(n_nodes, np.int64)
    rank[order] = np.arange(n_nodes)
    core_of = rank % n_cores
    lpos_of = rank // n_cores

    core_real = int(np.ceil(n_nodes / n_cores))
    NB = (core_real + P - 1) // P
    LOCAL = NB * P
    CR = LOCAL + 64            # core's table rows (dummies + pad at CR-1)
    WROWS = 2 * CR             # window = two cores
    TBL = n_cores * CR
    assert WROWS <= 32767, WROWS
    PADREL = CR - 1            # pad row, relative to window start
    ZROW = LOCAL               # zero row in partial tables
    PART_ROWS = LOCAL + P

    tid_of = core_of * CR + lpos_of          # node -> table id
    w_of_tid = tid_of // WROWS               # node -> window
    rel_of_tid = tid_of % WROWS

    NG = (NB + GB - 1) // GB
    gsizes = [min(GB, NB - g * GB) for g in range(NG)]
    while NB % CB != 0:
        CB -= 1
    NCB = NB // CB

    # ---- per-core edge structures ----
    ecore = core_of[dst]
    cores = []
    c_all = np.zeros((n_cores, NWIN, LOCAL), np.int32)
    for c in range(n_cores):
        m = ecore == c
        d_l = lpos_of[dst[m]]
        w = w_of_tid[src[m]]
        rel = rel_of_tid[src[m]]
        key = w * LOCAL + d_l
        o = np.argsort(key, kind="stable")
        key, rel_s = key[o], rel[o]
        cnt = np.bincount(key, minlength=NWIN * LOCAL).reshape(NWIN, LOCAL)
        c_all[c] = cnt
        starts = np.zeros(NWIN * LOCAL + 1, np.int64)
        np.cumsum(cnt.ravel(), out=starts[1:])
        cores.append(dict(rel_s=rel_s, starts=starts, cnt=cnt))

    # region orders per core (desc by count)
    for c in range(n_cores):
        cd = cores[c]
        cd["order_k"] = []
        cd["rank_k"] = []
        for k in range(NWIN):
            ok = np.argsort(-cd["cnt"][k], kind="stable")
            rk = np.empty(LOCAL, np.int64)
            rk[ok] = np.arange(LOCAL)
            cd["order_k"].append(ok)
            cd["rank_k"].append(rk)

    # global slot schedule S[k][g] (max over cores)
    S = np.zeros((NWIN, NG), np.int32)
    for k in range(NWIN):
        for c in range(n_cores):
            cd = cores[c]
            sc = cd["cnt"][k][cd["order_k"][k]]  # descending
            for g in range(NG):
                S[k, g] = max(S[k, g], int(sc[P * GB * g]))
    p.S = S

    # ---- per-core index arrays ----
    SCH = 4  # slot columns per compute chunk
    slot_cols = []   # (k, g, s0, sw, col_off, ncols, num_idx) ; one gather per column
    xr_cols = []
    cb_cols = []
    off = 0
    for k in range(NWIN):
        for g in range(NG):
            if S[k, g] == 0:
                continue
            for s0 in range(0, int(S[k, g]), SCH):
                sw = min(SCH, int(S[k, g]) - s0)
                n = P * gsizes[g] * sw
                slot_cols.append((k, g, s0, sw, off, n // 16, n))
                off += n // 16
    SLOT_W = off
    p.SCH = SCH
    off = 0
    for k in range(NWIN):
        for g in range(NG):
            if S[k, g] == 0:
                continue
            n = P * gsizes[g]
            xr_cols.append((k, g, off, n // 16, n))
            off += n // 16
    XR_W = off
    off = 0
    for gg in range(NCB):
        for k in range(NWIN):
            n = P * CB
            cb_cols.append((gg, k, off, n // 16, n))
            off += n // 16
    CB_W = off

    slot_idx = np.zeros((n_cores, 128, SLOT_W), np.int16)
    xr_idx = np.zeros((n_cores, 128, XR_W), np.int16)
    cb_idx = np.zeros((n_cores, 128, CB_W), np.int16)
    for c in range(n_cores):
        cd = cores[c]
        for (k, g, s0, sw, o, ncol, n) in slot_cols:
            nb = gsizes[g]
            dsts = cd["order_k"][k][P * GB * g: P * GB * g + P * nb]  # [nb*128]
            cnts = cd["cnt"][k][dsts]
            base = cd["starts"][k * LOCAL + dsts]
            s_ar = np.arange(s0, s0 + sw)[None, :]
            mask = s_ar < cnts[:, None]
            gat = np.where(mask, np.minimum(base[:, None] + s_ar, len(cd["rel_s"]) - 1), 0)
            vals = np.where(mask, cd["rel_s"][gat], PADREL)  # [nb*128, sw]
            vals = vals.reshape(nb, P, sw).transpose(2, 0, 1).ravel()  # (s,b,p)
            slot_idx[c, :, o:o + ncol] = _wrap16(vals)
        for (k, g, o, ncol, n) in xr_cols:
            nb = gsizes[g]
            dsts = cd["order_k"][k][P * GB * g: P * GB * g + P * nb]
            xr_idx[c, :, o:o + ncol] = _wrap16(dsts)
        for (gg, k, o, ncol, n) in cb_cols:
            d = np.arange(P * CB) + P * CB * gg  # base-order dst locals
            rk = cd["rank_k"][k][d]
            grp = rk // (P * GB)
            vals = np.where(S[k, np.minimum(grp, NG - 1)] > 0, rk, ZROW)
            cb_idx[c, :, o:o + ncol] = _wrap16(vals)

    p.P, p.NWIN, p.NB, p.LOCAL, p.CR, p.WROWS, p.TBL = P, NWIN, NB, LOCAL, CR, WROWS, TBL
    p.PADREL, p.ZROW, p.PART_ROWS = PADREL, ZROW, PART_ROWS
    p.GB, p.CB, p.NG, p.NCB, p.gsizes = GB, CB, NG, NCB, gsizes
    p.slot_cols, p.xr_cols, p.cb_cols = slot_cols, xr_cols, cb_cols
    p.SLOT_W, p.XR_W, p.CB_W = SLOT_W, XR_W, CB_W
    p.slot_idx, p.xr_idx, p.cb_idx = slot_idx, xr_idx, cb_idx
    p.tid_of, p.core_of, p.lpos_of = tid_of, core_of, lpos_of
    p.n_nodes, p.n_cores = n_nodes, n_cores
    return p


def build_inputs(p, x, W1l, b1l, W1r, b1r, att1, bias1,
                 W2l, b2l, W2r, b2r, att2, bias2):
    """Build the 8 per-core input dicts."""
    F_IN = x.shape[1]
    HID = W1l.shape[1]
    NCLS = W2l.shape[1]
    x = np.asarray(x, np.float32)
    xT = np.zeros((F_IN, p.TBL), np.float32)
    xT[:, p.tid_of] = x.T
    pad1 = np.tile((-PADC * np.sign(att1)).astype(np.float32)[None, :], (8, 1))
    ident_np = np.eye(128, dtype=np.float32)
    pad2 = np.zeros((1, 2 * NCLS), np.float32)
    pad2[0, :NCLS] = -PADC * np.sign(att2)
    base = {
        "xT": xT,
        "W1l": np.asarray(W1l, np.float32), "W1r": np.asarray(W1r, np.float32),
        "b1l_r": np.broadcast_to(np.asarray(b1l, np.float32), (128, HID)).copy(),
        "b1r_r": np.broadcast_to(np.asarray(b1r, np.float32), (128, HID)).copy(),
        "att1_r": np.broadcast_to(np.asarray(att1, np.float32), (128, HID)).copy(),
        "bias1_r": np.broadcast_to(np.asarray(bias1, np.float32), (128, HID)).copy(),
        "W2l": np.asarray(W2l, np.float32), "W2r": np.asarray(W2r, np.float32),
        "b2l_r": np.broadcast_to(np.asarray(b2l, np.float32), (128, NCLS)).copy(),
        "b2r_r": np.broadcast_to(np.asarray(b2r, np.float32), (128, NCLS)).copy(),
        "att2_r": np.broadcast_to(np.asarray(att2, np.float32), (128, NCLS)).copy(),
        "bias2_r": np.broadcast_to(np.asarray(bias2, np.float32), (128, NCLS)).copy(),
        "pad1": pad1, "pad2": pad2, "ident": ident_np,
    }
    ins = []
    for c in range(p.n_cores):
        m = base.copy()
        # core's dst shard, local order, transposed
        xTs = np.zeros((F_IN, p.LOCAL), np.float32)
        mine = p.core_of == c
        xTs[:, p.lpos_of[mine]] = x[mine].T
        m["xTs"] = xTs
        m["slot_idx"] = p.slot_idx[c]
        m["xr_idx"] = p.xr_idx[c]
        m["cb_idx"] = p.cb_idx[c]
        ins.append(m)
    return ins


def build_nc(p, F_IN=128, HID=64, NCLS=32, no_collective=False, stop_after=None, slot_parts=15):
    nc = bacc.Bacc()
    P = 128
    NWIN, NB, LOCAL, CR, WROWS, TBL = p.NWIN, p.NB, p.LOCAL, p.CR, p.WROWS, p.TBL
    GB, CB, NG, NCB = p.GB, p.CB, p.NG, p.NCB
    H2 = 2 * NCLS  # padded row width of layer-2 tables (256B)

    dp = nc.declare_dram_parameter
    xT = dp("xT", [F_IN, TBL], F32, isOutput=False)
    xTs = dp("xTs", [F_IN, LOCAL], F32, isOutput=False)
    W1l = dp("W1l", [F_IN, HID], F32, isOutput=False)
    W1r = dp("W1r", [F_IN, HID], F32, isOutput=False)
    b1l_r = dp("b1l_r", [P, HID], F32, isOutput=False)
    b1r_r = dp("b1r_r", [P, HID], F32, isOutput=False)
    att1_r = dp("att1_r", [P, HID], F32, isOutput=False)
    bias1_r = dp("bias1_r", [P, HID], F32, isOutput=False)
    W2l = dp("W2l", [HID, NCLS], F32, isOutput=False)
    W2r = dp("W2r", [HID, NCLS], F32, isOutput=False)
    b2l_r = dp("b2l_r", [P, NCLS], F32, isOutput=False)
    b2r_r = dp("b2r_r", [P, NCLS], F32, isOutput=False)
    att2_r = dp("att2_r", [P, NCLS], F32, isOutput=False)
    bias2_r = dp("bias2_r", [P, NCLS], F32, isOutput=False)
    pad1 = dp("pad1", [8, HID], F32, isOutput=False)
    pad2 = dp("pad2", [1, H2], F32, isOutput=False)
    ident_d = dp("ident", [128, 128], F32, isOutput=False)
    slot_idx_d = dp("slot_idx", [P, p.SLOT_W], I16, isOutput=False)
    xr_idx_d = dp("xr_idx", [P, p.XR_W], I16, isOutput=False)
    cb_idx_d = dp("cb_idx", [P, p.CB_W], I16, isOutput=False)
    out2_d = dp("out2", [LOCAL, NCLS], F32, isOutput=True)

    xl_tab = nc.dram_tensor("xl_tab", [TBL, HID], F32)
    xr_tab = nc.dram_tensor("xr_tab", [LOCAL, HID], F32)
    part1 = nc.dram_tensor("part1", [NWIN, p.PART_ROWS, 2 * HID], F32)
    hl_loc = nc.dram_tensor("hl_loc", [CR, H2], F32)
    hl_tab = nc.dram_tensor("hl_tab", [TBL, H2], F32, addr_space="Shared")
    hr_tab = nc.dram_tensor("hr_tab", [LOCAL, H2], F32)
    part2 = nc.dram_tensor("part2", [NWIN, p.PART_ROWS, 2 * NCLS], F32)

    with TileContext(nc) as tc:
        with (
            tc.tile_pool(name="const", bufs=1) as cpool,
            tc.tile_pool(name="xa", bufs=3) as xapool,
            tc.tile_pool(name="pa", bufs=2, space="PSUM") as papool,
            tc.tile_pool(name="ga", bufs=2) as gpool,
            tc.tile_pool(name="wk", bufs=2) as wpool,
            tc.tile_pool(name="sm", bufs=4) as spool,
            tc.tile_pool(name="cmb", bufs=2) as cbpool,
            tc.tile_pool(name="ph", bufs=3) as phpool,
        ):
            # ---- constants ----
            W1l_s = cpool.tile([F_IN, HID], F32); nc.sync.dma_start(out=W1l_s[:], in_=W1l[:])
            W1r_s = cpool.tile([F_IN, HID], F32); nc.sync.dma_start(out=W1r_s[:], in_=W1r[:])
            b1l_s = cpool.tile([P, HID], F32); nc.sync.dma_start(out=b1l_s[:], in_=b1l_r[:])
            b1r_s = cpool.tile([P, HID], F32); nc.sync.dma_start(out=b1r_s[:], in_=b1r_r[:])
            att1_s = cpool.tile([P, HID], F32); nc.sync.dma_start(out=att1_s[:], in_=att1_r[:])
            bias1_s = cpool.tile([P, HID], F32); nc.sync.dma_start(out=bias1_s[:], in_=bias1_r[:])
            W2l_s = cpool.tile([HID, NCLS], F32); nc.sync.dma_start(out=W2l_s[:], in_=W2l[:])
            W2r_s = cpool.tile([HID, NCLS], F32); nc.sync.dma_start(out=W2r_s[:], in_=W2r[:])
            b2l_s = cpool.tile([P, NCLS], F32); nc.sync.dma_start(out=b2l_s[:], in_=b2l_r[:])
            b2r_s = cpool.tile([P, NCLS], F32); nc.sync.dma_start(out=b2r_s[:], in_=b2r_r[:])
            att2_s = cpool.tile([P, NCLS], F32); nc.sync.dma_start(out=att2_s[:], in_=att2_r[:])
            bias2_s = cpool.tile([P, NCLS], F32); nc.sync.dma_start(out=bias2_s[:], in_=bias2_r[:])
            ident = cpool.tile([P, P], F32); nc.sync.dma_start(out=ident[:], in_=ident_d[:])
            h_all = cpool.tile([P, NB * HID], F32)
            zrow = cpool.tile([1, 2 * HID], F32)
            nc.vector.memset(zrow[:], 0.0)

            # ---- phase A: xl table (all rows), xr table (local) ----
            TB = 4  # tiles per batch
            for j in range(0, TBL // P, TB):
                nt = min(TB, TBL // P - j)
                xt = xapool.tile([P, TB * P], F32, tag="xt")
                nc.sync.dma_start(out=xt[:, :nt * P], in_=xT[:, j * P:(j + nt) * P])
                ot = xapool.tile([P, TB * HID], F32, tag="ot")
                for b in range(nt):
                    ps = papool.tile([P, HID], F32, tag="psA")
                    nc.tensor.matmul(out=ps[:], lhsT=xt[:, b * P:(b + 1) * P],
                                     rhs=W1l_s[:], start=True, stop=True)
                    nc.vector.tensor_tensor(out=ot[:, b * HID:(b + 1) * HID],
                                            in0=ps[:], in1=b1l_s[:], op=AL.add)
                nc.sync.dma_start(
                    out=xl_tab[j * P:(j + nt) * P, :].rearrange("(b q) f -> q b f", q=P),
                    in_=ot[:, :nt * HID].rearrange("q (b f) -> q b f", f=HID))
            # pad rows
            p1t = cpool.tile([8, HID], F32)
            nc.sync.dma_start(out=p1t[:], in_=pad1[:])
            nc.sync.dma_start(
                out=xl_tab[:].rearrange("(c r) f -> c r f", r=CR)[:, CR - 1, :],
                in_=p1t[:])
            for j in range(0, LOCAL // P, TB):
                nt = min(TB, LOCAL // P - j)
                xt = xapool.tile([P, TB * P], F32, tag="xt")
                nc.sync.dma_start(out=xt[:, :nt * P], in_=xTs[:, j * P:(j + nt) * P])
                ot = xapool.tile([P, TB * HID], F32, tag="ot")
                for b in range(nt):
                    ps = papool.tile([P, HID], F32, tag="psA")
                    nc.tensor.matmul(out=ps[:], lhsT=xt[:, b * P:(b + 1) * P],
                                     rhs=W1r_s[:], start=True, stop=True)
                    nc.vector.tensor_tensor(out=ot[:, b * HID:(b + 1) * HID],
                                            in0=ps[:], in1=b1r_s[:], op=AL.add)
                nc.sync.dma_start(
                    out=xr_tab[j * P:(j + nt) * P, :].rearrange("(b q) f -> q b f", q=P),
                    in_=ot[:, :nt * HID].rearrange("q (b f) -> q b f", f=HID))

            # zero rows of partial tables
            for k in range(NWIN):
                nc.sync.dma_start(out=part1[k, p.ZROW:p.ZROW + 1, :], in_=zrow[:])
                nc.sync.dma_start(out=part2[k, p.ZROW:p.ZROW + 1, :], in_=zrow[:, :2 * NCLS])

            # ---- slot phase (shared for both layers) ----
            def slot_phase(tab, tab_w, rtab, F, att_s, part):
                for g in range(NG):
                    nb = p.gsizes[g]
                    for k in range(NWIN):
                        if p.S[k, g] == 0:
                            continue
                        Sg = int(p.S[k, g])
                        (ko, go, xoff, xncol, xn) = next(
                            t for t in p.xr_cols if t[0] == k and t[1] == g)
                        xidx = spool.tile([P, 64], I16, tag="xidx")
                        nc.sync.dma_start(out=xidx[:, :xncol], in_=xr_idx_d[:, xoff:xoff + xncol])
                        gr = wpool.tile([P, GB * 64], F32, tag="gr")
                        if slot_parts & 1:
                            nc.gpsimd.dma_gather(
                                out_ap=gr[:, :nb * tab_w].rearrange("q (c f) -> q c f", f=tab_w),
                                in_ap=rtab[:, :],
                                idxs_ap=xidx[:, :xncol],
                                num_idxs=xn, num_idxs_reg=xn, elem_size=tab_w)
                        else:
                            nc.vector.memset(gr[:], 0.0)
                        sk = spool.tile([P, GB], F32, tag="sk")
                        wk = wpool.tile([P, GB * 64], F32, tag="wk")
                        chunks = [t for t in p.slot_cols if t[0] == k and t[1] == g]
                        for ci, (kc, gc, s0, sw, off, ncol, n) in enumerate(chunks):
                            nch = nb * sw
                            sidx = spool.tile([P, 64 * p.SCH], I16, tag="sidx")
                            nc.sync.dma_start(out=sidx[:, :ncol], in_=slot_idx_d[:, off:off + ncol])
                            gx = gpool.tile([P, GB * p.SCH * 64], F32, tag="gx")
                            ccol = ncol // sw  # idx cols per column-call
                            ncall = n // sw
                            for s in range(sw):
                                nc.gpsimd.dma_gather(
                                    out_ap=gx[:, s * nb * tab_w:(s + 1) * nb * tab_w]
                                        .rearrange("q (c f) -> q c f", f=tab_w),
                                    in_ap=tab[k * WROWS:(k + 1) * WROWS, :],
                                    idxs_ap=sidx[:, s * ccol:(s + 1) * ccol],
                                    num_idxs=ncall, num_idxs_reg=ncall, elem_size=tab_w)
                            # layout: [q, sw, nb, tab_w]; use feature slice 0:F
                            gx4 = gx[:, :nch * tab_w].rearrange(
                                "q (s b f) -> q s b f", s=sw, b=nb)[:, :, :, 0:F]
                            t = wpool.tile([P, GB * p.SCH * 64], F32, tag="t")
                            t4 = t[:, :nch * F].rearrange("q (s b f) -> q s b f", s=sw, b=nb)
                            gr4 = gr[:, :nb * tab_w].rearrange("q (b f) -> q b f", b=nb)[:, :, 0:F]
                            gr4 = gr4.unsqueeze(1).to_broadcast([P, sw, nb, F])
                            nc.vector.tensor_tensor(out=t4, in0=gx4, in1=gr4, op=AL.add)
                            nc.vector.scalar_tensor_tensor(
                                out=t[:, :nch * F], in0=t[:, :nch * F], scalar=0.2,
                                in1=t[:, :nch * F], op0=AL.mult, op1=AL.max)
                            attb = att_s[:].unsqueeze(1).to_broadcast([P, nch, F])
                            nc.vector.tensor_tensor(
                                out=t[:, :nch * F].rearrange("q (c f) -> q c f", f=F),
                                in0=t[:, :nch * F].rearrange("q (c f) -> q c f", f=F),
                                in1=attb, op=AL.mult)
                            e = spool.tile([P, GB * p.SCH], F32, tag="e")
                            nc.vector.reduce_sum(
                                out=e[:, :nch],
                                in_=t[:, :nch * F].rearrange("q (c f) -> q c f", f=F),
                                axis=AX.X)
                            ex = spool.tile([P, GB * p.SCH], F32, tag="ex")
                            nc.scalar.activation(out=ex[:, :nch], in_=e[:, :nch], func=AF.Exp)
                            sk_c = spool.tile([P, GB], F32, tag="sk_c")
                            nc.vector.reduce_sum(
                                out=sk_c[:, :nb],
                                in_=ex[:, :nch].rearrange("q (s b) -> q b s", s=sw),
                                axis=AX.X)
                            exb = ex[:, :nch].unsqueeze(2).to_broadcast([P, nch, F])
                            nc.vector.tensor_tensor(
                                out=t[:, :nch * F].rearrange("q (c f) -> q c f", f=F),
                                in0=gx4.rearrange("q s b f -> q (s b) f"),
                                in1=exb, op=AL.mult)
                            wk_c = wpool.tile([P, GB * 64], F32, tag="wk_c")
                            nc.vector.reduce_sum(
                                out=wk_c[:, :nb * F],
                                in_=t[:, :nch * F].rearrange("q (s b f) -> q b f s", s=sw, b=nb),
                                axis=AX.X)
                            if ci == 0:
                                nc.vector.tensor_copy(out=sk[:, :nb], in_=sk_c[:, :nb])
                                nc.vector.tensor_copy(out=wk[:, :nb * F], in_=wk_c[:, :nb * F])
                            else:
                                nc.vector.tensor_tensor(out=sk[:, :nb], in0=sk[:, :nb],
                                                        in1=sk_c[:, :nb], op=AL.add)
                                nc.vector.tensor_tensor(out=wk[:, :nb * F], in0=wk[:, :nb * F],
                                                        in1=wk_c[:, :nb * F], op=AL.add)
                        rows0 = P * GB * g
                        nrows = P * nb
                        if not (slot_parts & 8):
                            continue
                        nc.sync.dma_start(
                            out=part[k, rows0:rows0 + nrows, 0:F]
                                .rearrange("(b q) f -> q b f", q=P),
                            in_=wk[:, :nb * F].rearrange("q (b f) -> q b f", f=F))
                        nc.sync.dma_start(
                            out=part[k, rows0:rows0 + nrows, F:F + 1]
                                .rearrange("(b q) f -> q b f", q=P),
                            in_=sk[:, :nb].unsqueeze(2))

            # ---- combine phase (shared) ----
            def combine_phase(part, F, Fr, bias_s, relu, sink):
                # Fr = row width of partial table (2*F); sink(gg, res_tile)
                for gg in range(NCB):
                    ct = cbpool.tile([P, NWIN * CB * Fr], F32, tag="ct")
                    for k in range(NWIN):
                        (ggo, ko, off, ncol, n) = next(
                            t for t in p.cb_cols if t[0] == gg and t[1] == k)
                        cidx = spool.tile([P, 64], I16, tag="cidx")
                        nc.sync.dma_start(out=cidx[:, :ncol], in_=cb_idx_d[:, off:off + ncol])
                        nc.gpsimd.dma_gather(
                            out_ap=ct[:, k * CB * Fr:(k + 1) * CB * Fr]
                                .rearrange("q (c f) -> q c f", f=Fr),
                            in_ap=part[k, :, :],
                            idxs_ap=cidx[:, :ncol],
                            num_idxs=n, num_idxs_reg=n, elem_size=Fr)
                    tw = cbpool.tile([P, CB * Fr], F32, tag="tw")
                    nc.vector.tensor_tensor(out=tw[:], in0=ct[:, 0:CB * Fr],
                                            in1=ct[:, CB * Fr:2 * CB * Fr], op=AL.add)
                    nc.vector.tensor_tensor(out=tw[:], in0=tw[:],
                                            in1=ct[:, 2 * CB * Fr:3 * CB * Fr], op=AL.add)
                    nc.vector.tensor_tensor(out=tw[:], in0=tw[:],
                                            in1=ct[:, 3 * CB * Fr:4 * CB * Fr], op=AL.add)
                    tw3 = tw[:].rearrange("q (c f) -> q c f", f=Fr)
                    sden = spool.tile([P, CB], F32, tag="sden")
                    nc.vector.tensor_scalar(out=sden[:], in0=tw3[:, :, F:F + 1].squeeze(2),
                                            scalar1=1e-16, scalar2=None, op0=AL.add)
                    rr = spool.tile([P, CB], F32, tag="rr")
                    nc.vector.reciprocal(out=rr[:], in_=sden[:])
                    res = phpool.tile([P, CB * F], F32, tag="res")
                    res3 = res[:].rearrange("q (c f) -> q c f", f=F)
                    rrb = rr[:].unsqueeze(2).to_broadcast([P, CB, F])
                    nc.vector.tensor_tensor(out=res3, in0=tw3[:, :, 0:F], in1=rrb, op=AL.mult)
                    bb = bias_s[:].unsqueeze(1).to_broadcast([P, CB, F])
                    nc.vector.tensor_tensor(out=res3, in0=res3, in1=bb, op=AL.add)
                    if relu:
                        nc.scalar.activation(out=res[:], in_=res[:], func=AF.Relu)
                    sink(gg, res)

            # ===== layer 1 =====
            go = stop_after is None
            if go or stop_after in ("slots1", "combine1", "ag"):
                slot_phase(xl_tab, HID, xr_tab, HID, att1_s, part1)

            def sink1(gg, res):
                nc.vector.tensor_copy(out=h_all[:, gg * CB * HID:(gg + 1) * CB * HID],
                                      in_=res[:])
            if go or stop_after in ("combine1", "ag"):
                combine_phase(part1, HID, 2 * HID, bias1_s, True, sink1)

            # ===== hl / hr  =====
            for j in range(NB if (go or stop_after == "ag") else 0):
                pst = papool.tile([HID, P], F32, tag="psT")
                nc.tensor.transpose(out=pst[:], in_=h_all[:, j * HID:(j + 1) * HID],
                                    identity=ident[:])
                hT = phpool.tile([HID, P], F32, tag="hT")
                nc.vector.tensor_copy(out=hT[:], in_=pst[:])
                psl = papool.tile([P, NCLS], F32, tag="psl")
                nc.tensor.matmul(out=psl[:], lhsT=hT[:], rhs=W2l_s[:], start=True, stop=True)
                hlb = phpool.tile([P, NCLS], F32, tag="hlb")
                nc.vector.tensor_tensor(out=hlb[:], in0=psl[:], in1=b2l_s[:], op=AL.add)
                nc.sync.dma_start(out=hl_loc[j * P:(j + 1) * P, 0:NCLS], in_=hlb[:])
                psr = papool.tile([P, NCLS], F32, tag="psr")
                nc.tensor.matmul(out=psr[:], lhsT=hT[:], rhs=W2r_s[:], start=True, stop=True)
                hrb = phpool.tile([P, NCLS], F32, tag="hrb")
                nc.vector.tensor_tensor(out=hrb[:], in0=psr[:], in1=b2r_s[:], op=AL.add)
                nc.sync.dma_start(out=hr_tab[j * P:(j + 1) * P, 0:NCLS], in_=hrb[:])
            if go or stop_after == "ag":
                p2t = cpool.tile([1, H2], F32)
                nc.sync.dma_start(out=p2t[:], in_=pad2[:])
                nc.sync.dma_start(out=hl_loc[CR - 1:CR, :], in_=p2t[:])
            # fill unused cols of hl_loc rows? not needed: gathers only read
            # rows that were written (real srcs + pad row); cols F:2F are read
            # by the gather but never consumed by compute.
            if go or stop_after == "ag":
                if no_collective:
                    # debug: copy own shard into window 0 slot (wrong results)
                    nc.sync.dma_start(out=hl_tab[0:CR, :], in_=hl_loc[:, :])
                else:
                    nc.gpsimd.collective_compute(
                        "AllGather", AL.bypass,
                        replica_groups=[list(range(p.n_cores))],
                        ins=[hl_loc[:, :]], outs=[hl_tab[:, :]])

            # ===== layer 2 =====
            if go:
                slot_phase(hl_tab, H2, hr_tab, NCLS, att2_s, part2)

            def sink2(gg, res):
                nc.sync.dma_start(
                    out=out2_d[gg * CB * P:(gg + 1) * CB * P, :]
                        .rearrange("(b q) f -> q b f", q=P),
                    in_=res[:].rearrange("q (b f) -> q b f", f=NCLS))
            if go:
                combine_phase(part2, NCLS, 2 * NCLS, bias2_s, False, sink2)
            else:
                # bisect mode: still write the output so the NEFF has it
                zo = phpool.tile([P, CB * NCLS], F32, tag="res")
                nc.vector.memset(zo[:], 0.0)
                sink2(0, zo)

    nc.compile()
    return nc


def unshard_output(p, results):
    NCLS = results[0]["out2"].shape[1]
    out = np.zeros((p.n_nodes, NCLS), np.float32)
    for c in range(p.n_cores):
        mine = np.where(p.core_of == c)[0]
        out[mine] = results[c]["out2"][p.lpos_of[mine]]
    return out


# ============================================================================
# Harness entry point: full inputs in, full output out.
# ============================================================================
_NC_CACHE = {}


def _get_nc(p):
    key = (tuple(p.S.ravel().tolist()), p.NB, p.TBL, p.SLOT_W, p.XR_W, p.CB_W)
    if key not in _NC_CACHE:
        _NC_CACHE.clear()
        _NC_CACHE[key] = build_nc(p)
    return _NC_CACHE[key]


def kernel(x, edge_index, W1l, b1l, W1r, b1r, att1, bias1,
           W2l, b2l, W2r, b2r, att2, bias2):
    from concourse.bass_utils import run_bass_kernel_spmd
    x = np.asarray(x)
    edge_index = np.asarray(edge_index)
    p = build_plan(edge_index, x.shape[0])
    ins = build_inputs(p, x, W1l, b1l, W1r, b1r, att1, bias1,
                       W2l, b2l, W2r, b2r, att2, bias2)
    nc = _get_nc(p)
    res = run_bass_kernel_spmd(nc, ins, core_ids=list(range(p.n_cores)))
    return unshard_output(p, [res.results[c] for c in range(p.n_cores)])

